# revision 3
# baseline (speedup 1.0000x reference)
"""Blake2 soft-cipher Bass kernel v2 for Trainium2 (8 NeuronCores, data parallel).

Key reductions vs the straightforward implementation (all validated against
the f32 reference numerically):
  - rot16/24/32 on soft_xor outputs (>= 0.0132 > 2^-9) have identically-zero
    wrapped terms in f32, so they are exact scales by 2^-k.
  - A rot32-scaled value (<= 2.4e-10) is invisible to f32 adds with O(1)
    values and makes its soft_xor x-side sigmoid exactly sigmoid(-5); the
    whole first soft_xor of each G is dead code.
  - Dropping the d-term (<= 1.5e-5 influence) makes the b/c/d lineages
    compile-time constants: only the a-lineage (soft_add chain through the
    message words) runs on hardware.  Measured rel err ~1.8e-3 in f32,
    ~8.4e-3 in fp16 (gate: 2e-2).
  - soft_xor with a constant operand is a quadratic A*ys^2 + B*ys + C in the
    other operand's sigmoid.
Engine split: ACT does the 9 packed sigmoids per round, DVE the packed fp16
tensor ops (2x/4x perf modes), Pool (gpssimd) the per-lane quad STTs.
"""
import sys
sys.path.insert(0, "/opt/trn_rl_repo")
import math
import numpy as np
from concourse import bass, mybir
from concourse.tile import TileContext
from concourse.bass_primitives_rust import SemaphoreHandle
from concourse.bass import _bass_rust

A = mybir.AluOpType
F = mybir.ActivationFunctionType

# ---------------------------------------------------------------- geometry
P = 128
import os as _os
FD = int(_os.environ.get("K_FD", "652"))
CHUNKS = int(_os.environ.get("K_CHUNKS", "3"))
LANES = 4
CHUNK_ROWS = P * FD       # 125,056
CORE_ROWS = CHUNK_ROWS * CHUNKS   # 250,112
N_CORES = 8
TOTAL_ROWS = 2_000_000
PAD_ROWS = CORE_ROWS * N_CORES    # 2,000,896

DT = mybir.dt.float16     # on-chip compute dtype
NPDT = np.float16
DT32 = mybir.dt.float32

_IV_INTS = [7640891576956012808, 13503953896175478587, 4354685564936845355,
            11912009170470909681, 5840696475078001361, 11170449401992604703,
            2270897969802886507, 6620516959819538809]
IV = (np.asarray(_IV_INTS, dtype=np.float32) / np.float32(2.0**64)).astype(np.float32)
ROUNDS = 10

f32 = np.float32


# ------------------------------------------------------- build-time consts
def _sig(z):
    return f32(1.0 / (1.0 + math.exp(-float(z))))


def _sa(x, y):  # const soft_add (f32 reference semantics)
    s = f32(f32(x) + f32(y))
    w = _sig(f32(f32(10.0) * f32(s - f32(1.0))))
    return f32(s - w)


def _sa0(x):  # soft_add with dropped tiny second operand
    x = f32(x)
    w = _sig(f32(f32(10.0) * f32(x - f32(1.0))))
    return f32(x - w)


ALPHA = _sig(-5.0)
QA = f32(float(ALPHA) * (1.0 - float(ALPHA)))
QB = f32((1.0 - 2.0 * float(ALPHA)) - float(QA))
QC = ALPHA


def _quad_alpha(xs):  # soft_xor(tiny-const, y) as quadratic in ys
    xs = float(xs)
    return f32(float(QA) * xs * xs + float(QB) * xs + float(QC))


def _rot63c(x):
    x = f32(x)
    return f32(f32(2.0) * x - (f32(1.0) if x >= f32(0.5) else f32(0.0)))


def build_consts():
    """Per-round constants. The b/c lineages restart from IV each round, so
    rounds 1..9 share one constant set; round 0 differs only in b-entry."""
    # G1 c lineage (c entry = IV[i] every round)
    vc2_g1 = [_sa0(_sa0(IV[i])) for i in range(4)]
    bout_g1 = []  # G1 b-outputs, lane i
    for i in range(4):
        xs = _sig(f32(f32(10.0) * f32(vc2_g1[i] - f32(0.5))))
        bout_g1.append(_rot63c(_quad_alpha(xs)))
    # G2 c lineage: lane k reads gamma = vc2_g1[(k+2)%4]
    vc2_g2 = [_sa0(_sa0(vc2_g1[(k + 2) % 4])) for k in range(4)]
    # final-xor quad coeffs: state[j] pairs with c at v[8+j] = G2 lane (j+2)%4
    cfinal = [vc2_g2[(j + 2) % 4] for j in range(4)]
    alphac = [_sig(f32(f32(10.0) * f32(cfinal[j] - f32(0.5)))) for j in range(4)]
    AJ = [f32(float(a) * (1.0 - float(a))) for a in alphac]
    BJ = [f32((1.0 - 2.0 * float(a)) - float(aj)) for a, aj in zip(alphac, AJ)]
    CJ = alphac
    # G2 b-outputs land at v[4 + (k+1)%4]
    bout_g2pos = [None] * 4
    for k in range(4):
        xs = _sig(f32(f32(10.0) * f32(vc2_g2[k] - f32(0.5))))
        bout_g2pos[(k + 1) % 4] = _rot63c(_quad_alpha(xs))
    # next-round b-entries: state[4+j] = quad_alpha(sig(10(vb3-0.5)))
    state4 = [_quad_alpha(_sig(f32(f32(10.0) * f32(bout_g2pos[j] - f32(0.5)))))
              for j in range(4)]
    # round-0 G1 collapsed a-chain head: va1 = SA(IV[i], IV[4+i])
    va1c_r0 = [_sa(IV[i], IV[4 + i]) for i in range(4)]
    # b-entries: round 0 -> IV[4..7] (only via va1c_r0); rounds >=1 -> state4
    beta1 = state4                      # G1 b-entry, rounds 1..9
    beta2 = [bout_g1[(k + 1) % 4] for k in range(4)]  # G2 lane k b-entry (all rounds)
    return dict(va1c_r0=va1c_r0, beta1=beta1, beta2=beta2,
                AJ=AJ, BJ=BJ, CJ=CJ, out47=state4)


CONSTS = build_consts()


# ---------------------------------------------------------------- program
class Program:
    def __init__(self):
        self.nc = bass.Bass("TRN2")
        # running busy-ns estimates for flexible-op placement
        self.est = {"dve": 0.0, "act": 0.0, "pool": 0.0}

    # cost helpers (fp16, TRN2): DVE 0.96GHz, Pool 1.2GHz
    def _dve_tt(self, n):  # packed fp16 TT: 0.5 c/e
        return (0.5 * n + 58) / 0.96
    def _dve_ts(self, n):  # packed fp16 TS: 0.25 c/e
        return (0.25 * n + 58) / 0.96
    def _dve_stt(self, n):
        return (1.0 * n + 58) / 0.96
    def _pool_op(self, n, eff=0.6):
        return n / 1.2 / eff + 95
    def _act(self, n):
        return (n + 444 / 2) / 1.2

    # ---------- emitters
    def act_sig(self, out, in_, bias):
        bap = self.bias_m10 if bias == -10.0 else self.bias_m5
        self.nc.scalar.activation(out, in_, F.Sigmoid, bias=bap, scale=10.0)
        self.est["act"] += self._act(in_.shape[-1])

    def tt(self, out, a, b, op, pool_ok=False):
        import os
        if pool_ok is True and os.environ.get("K_POOLTT", "0") == "1":
            pool_now = True
        elif pool_ok == "force":
            pool_now = True
        else:
            pool_now = False
        if pool_now:
            self.nc.gpsimd.tensor_tensor(out, a, b, op=op)
            self.est["pool"] += self._pool_op(out.shape[-1], eff=0.42)
        else:
            self.nc.vector.tensor_tensor(out, a, b, op=op)
            self.est["dve"] += self._dve_tt(out.shape[-1])

    def ts(self, out, in0, s1, s2=None, op0=A.add, op1=None, rev0=False):
        if op1 is None:
            i = self.nc.vector.tensor_scalar(out, in0, float(s1), None, op0=op0)
        else:
            i = self.nc.vector.tensor_scalar(out, in0, float(s1), float(s2), op0=op0, op1=op1)
        if rev0:
            i.ins.reverse0 = True
        self.est["dve"] += self._dve_ts(out.shape[-1])


    # ---------- whole program
    def build(self):
        nc = self.nc
        C = CONSTS
        self.msgT = nc.declare_dram_parameter("msgT", [16, CORE_ROWS], DT, isOutput=False)
        self.outT = nc.declare_dram_parameter("outT", [8, CORE_ROWS], DT, isOutput=True)
        PK = LANES * FD  # packed width

        with TileContext(nc) as tc:
            with (
                tc.tile_pool(name="persist", bufs=1) as pp,
                tc.tile_pool(name="scrp", bufs=2) as sp,
            ):
                # message tiles: per chunk, mx = words 0,2,..,14  my = words 1,3,..,15
                # host pre-permutes msgT rows to [x words (0,2,..14), y words (1,3,..15)]
                # so each G-group's 4 words are one contiguous 4-row DMA.
                mx = [pp.tile([P, 8 * FD], DT, tag=f"mx{h}", name=f"mx{h}") for h in range(CHUNKS)]
                my = [pp.tile([P, 8 * FD], DT, tag=f"my{h}", name=f"my{h}") for h in range(CHUNKS)]
                def mdma(dst_ap, row0, off):
                    src = self.msgT[row0:row0 + 4, off:off + CHUNK_ROWS].rearrange(
                        "w (p f) -> p w f", p=P)
                    dst = dst_ap.rearrange("p (w f) -> p w f", w=4)
                    nc.sync.dma_start(out=dst, in_=src)
                # chunk 0's g1-x words split per word so lane TS ops can
                # start as soon as each word lands
                for k in range(4):
                    srcw = self.msgT[k:k + 1, 0:CHUNK_ROWS].rearrange(
                        "o (p f) -> p (o f)", p=P)
                    nc.sync.dma_start(out=mx[0][:][:, k * FD:(k + 1) * FD], in_=srcw)
                for h in range(1, CHUNKS):   # g1 x words for remaining chunks
                    mdma(mx[h][:][:, 0:4 * FD], 0, h * CHUNK_ROWS)
                for h in range(CHUNKS):   # then g1 y words
                    mdma(my[h][:][:, 0:4 * FD], 8, h * CHUNK_ROWS)
                for h in range(CHUNKS):   # then g2 x / y
                    mdma(mx[h][:][:, 4 * FD:8 * FD], 4, h * CHUNK_ROWS)
                for h in range(CHUNKS):
                    mdma(my[h][:][:, 4 * FD:8 * FD], 12, h * CHUNK_ROWS)

                # output tiles
                outd = [pp.tile([P, 4 * FD], DT, tag=f"outd{h}", name=f"outd{h}")
                        for h in range(CHUNKS)]
                outc = pp.tile([P, 4 * FD], DT, tag="outc", name="outc")
                for j in range(4):
                    nc.vector.memset(outc[:][:, j * FD:(j + 1) * FD], float(C["out47"][j]))
                for h in range(CHUNKS):
                    off = h * CHUNK_ROWS
                    dst = self.outT[4:8, off:off + CHUNK_ROWS].rearrange(
                        "w (p f) -> p w f", p=P)
                    nc.sync.dma_start(out=dst, in_=outc[:].rearrange("p (w f) -> p w f", w=4))
                bias_m10 = pp.tile([P, 1], DT, tag="bias_m10", name="bias_m10")
                bias_m5 = pp.tile([P, 1], DT, tag="bias_m5", name="bias_m5")
                nc.vector.memset(bias_m10[:], -10.0)
                nc.vector.memset(bias_m5[:], -5.0)
                self.bias_m10 = bias_m10[:]
                self.bias_m5 = bias_m5[:]

                # packed work tiles (rotating)
                def scr(tag, bufs=2):
                    return sp.tile([P, PK], DT, tag=tag, name=tag, bufs=bufs)[:]

                vs_next = [None] * CHUNKS  # holds s1 (pre-folded) for next round

                all_chunk_ops = [[] for _ in range(CHUNKS)]
                for rnd in range(ROUNDS):
                    per_chunk_ops = []
                    for h in range(CHUNKS):
                        ops = []
                        mxh, myh = mx[h][:], my[h][:]

                        def G_chain(s_in, mx_ap, my_ap, h=h, first_r0=False,
                                    tail="g2"):
                            """One G group's a-chain with pre-added message words:
                            s2 = (s1+mx) - w1 so the add overlaps the sigmoid.
                            tail='g1': ends producing s1 for G2 (beta2 pre-added);
                            tail='g2': ends with va4."""
                            seq = []
                            cell = {}
                            if first_r0:
                                # round0 G1: s2 = mx + va1c per lane
                                def f_s2(h=h, mx_ap=mx_ap):
                                    t = scr(f"s{h}")
                                    for i in range(4):
                                        self.ts(t[:, i * FD:(i + 1) * FD],
                                                mx_ap[:, i * FD:(i + 1) * FD],
                                                C["va1c_r0"][i], op0=A.add)
                                    cell["s2"] = t
                                seq.append(f_s2)
                            else:
                                def f_w1(h=h, s_in=s_in):
                                    if cell.get("s1") is None:
                                        cell["s1"] = s_in() if callable(s_in) else s_in
                                    t = scr(f"w{h}")
                                    self.act_sig(t, cell["s1"], -10.0)
                                    cell["w1"] = t
                                def f_mxx(h=h, mx_ap=mx_ap):
                                    t = scr(f"v{h}")
                                    self.tt(t, cell["s1"], mx_ap, A.add, pool_ok=True)
                                    cell["mxx"] = t
                                def f_s2(h=h):
                                    t = scr(f"s{h}")
                                    self.tt(t, cell["mxx"], cell["w1"], A.subtract)
                                    cell["s2"] = t
                                seq += [f_w1, f_mxx, f_s2]

                            def f_w2(h=h):
                                t = scr(f"w{h}")
                                self.act_sig(t, cell["s2"], -10.0)
                                cell["w2"] = t
                            def f_va2(h=h):
                                t = scr(f"v{h}")
                                self.tt(t, cell["s2"], cell["w2"], A.subtract)
                                cell["va2"] = t
                            def f_w6(h=h):
                                t = scr(f"w{h}")
                                self.act_sig(t, cell["va2"], -10.0)
                                cell["w6"] = t
                            def f_myy(h=h, my_ap=my_ap):
                                t = scr(f"s{h}")
                                self.tt(t, cell["va2"], my_ap, A.add, pool_ok=True)
                                cell["myy"] = t
                            def f_s7(h=h):
                                t = scr(f"v{h}")
                                self.tt(t, cell["myy"], cell["w6"], A.subtract)
                                cell["s7"] = t
                            def f_w7(h=h):
                                t = scr(f"w{h}")
                                self.act_sig(t, cell["s7"], -10.0)
                                cell["w7"] = t
                            seq += [f_w2, f_va2, f_w6, f_myy, f_s7, f_w7]
                            if tail == "g1":
                                # pre-add beta2 per lane (overlaps w7), then
                                # one packed subtract gives G2's s1 directly
                                def f_pre(h=h):
                                    t = scr(f"s{h}")
                                    for k in range(4):
                                        self.ts(t[:, k * FD:(k + 1) * FD],
                                                cell["s7"][:, k * FD:(k + 1) * FD],
                                                C["beta2"][k], op0=A.add)
                                    cell["pre"] = t
                                def f_out(h=h):
                                    t = scr(f"v{h}")
                                    self.tt(t, cell["pre"], cell["w7"], A.subtract)
                                    cell["va4"] = t  # actually s1 of G2
                                seq += [f_pre, f_out]
                            else:
                                def f_va4(h=h):
                                    t = scr(f"v{h}")
                                    self.tt(t, cell["s7"], cell["w7"], A.subtract)
                                    cell["va4"] = t
                                seq += [f_va4]
                            return seq, cell

                        # ---- G1 (produces G2's s1 with beta2 pre-folded)
                        s_in = (lambda h=h: vs_next[h])
                        g1_seq, g1_cell = G_chain(s_in, mxh[:, 0:4 * FD],
                                                  myh[:, 0:4 * FD],
                                                  first_r0=(rnd == 0), tail="g1")
                        ops += g1_seq

                        g2_seq, g2_cell = G_chain(None, mxh[:, 4 * FD:8 * FD],
                                                  myh[:, 4 * FD:8 * FD])
                        # patch: g2's s1 comes from g1's output at emission time
                        def f_fix_s1(g1_cell=g1_cell, g2_cell=g2_cell):
                            g2_cell["s1"] = g1_cell["va4"]
                        ops.append(f_fix_s1)
                        ops += g2_seq

                        # ---- final: xs = sig(10 va4' - 5); per-lane quad via
                        # Horner: r = (A*xs + B)*xs + C(+beta1 fold)
                        fin = {}
                        def f_xs(h=h, g2_cell=g2_cell, fin=fin):
                            t = scr(f"w{h}")
                            self.act_sig(t, g2_cell["va4"], -5.0)
                            fin["xs"] = t
                        def f_t2(h=h, fin=fin):
                            t = scr(f"v{h}")
                            x = fin["xs"]
                            for j in range(4):
                                self.ts(t[:, j * FD:(j + 1) * FD],
                                        x[:, j * FD:(j + 1) * FD],
                                        C["AJ"][j], C["BJ"][j], op0=A.mult, op1=A.add)
                            fin["t2"] = t
                        def f_p(h=h, fin=fin):
                            t = scr(f"s{h}")
                            self.tt(t, fin["t2"], fin["xs"], A.mult,
                                    pool_ok="force" if __import__("os").environ.get("K_POOLP", "0") == "1" else False)
                            fin["p"] = t
                        def f_out(h=h, rnd=rnd, fin=fin):
                            last = rnd == ROUNDS - 1
                            if last:
                                dst = outd[h][:]
                            else:
                                dst = scr(f"n{h}", bufs=1)
                            for j in range(4):
                                cadd = float(C["CJ"][j]) + (0.0 if last else float(C["beta1"][j]))
                                self.ts(dst[:, j * FD:(j + 1) * FD],
                                        fin["p"][:, j * FD:(j + 1) * FD],
                                        cadd, op0=A.add)
                            if not last:
                                vs_next[h] = dst
                        ops += [f_xs, f_t2, f_p, f_out]
                        per_chunk_ops.append(ops)

                    for h in range(CHUNKS):
                        all_chunk_ops[h] += per_chunk_ops[h]

                # global interleave with persistent phase skew between chunks
                _env = __import__("os").environ
                if "K_SKEWS" in _env:
                    _sk = [int(x) for x in _env["K_SKEWS"].split(",")]
                else:
                    _s = int(_env.get("K_SKEW", "5"))
                    _sk = [_s * h for h in range(CHUNKS)]
                lanes = [[None] * _sk[h] + list(o)
                         for h, o in enumerate(all_chunk_ops)]
                while lanes:
                    nxt = []
                    for l in lanes:
                        op = l.pop(0)
                        if op is not None:
                            op()
                        if l:
                            nxt.append(l)
                    lanes = nxt

                # ---- output DMAs (data columns, per lane so each fires as
                # soon as its final TS completes)
                for h in range(CHUNKS):
                    off = h * CHUNK_ROWS
                    for j in range(4):
                        dst = self.outT[j:j + 1, off:off + CHUNK_ROWS].rearrange(
                            "o (p f) -> p (o f)", p=P)
                        nc.sync.dma_start(out=dst, in_=outd[h][:][:, j * FD:(j + 1) * FD])
        hoist_excess_waits(nc)
        return nc


def hoist_excess_waits(nc, max_waits=1):
    """Walrus can't encode >~2 sync waits per instruction; move excess into
    standalone NoOps (1 wait each) right before the instruction."""
    n_hoisted = 0
    for fn in nc.m.functions:
        for blk in fn.blocks:
            need = False
            for inst in blk.instructions:
                si = inst.sync_info
                if si is not None and len(si.on_wait) > max_waits:
                    need = True
                    break
            if not need:
                continue
            newl = []
            for inst in blk.instructions:
                si = inst.sync_info
                if si is not None and len(si.on_wait) > max_waits:
                    conds = list(si.on_wait)
                    keep = conds[-max_waits:]
                    for c in conds[:-max_waits]:
                        nop = mybir.InstNoOp(
                            name=nc.get_next_instruction_name(), ins=[], outs=[])
                        nop.engine = inst.engine
                        _bass_rust.wait_op(
                            nop, SemaphoreHandle(c.ant_name, c.id),
                            c.wait_value, "sem-ge", False)
                        newl.append(nop)
                        n_hoisted += 1
                    inst.sync_info = mybir.SyncInfo(on_wait=keep, on_update=list(si.on_update))
                newl.append(inst)
            blk.instructions = newl
    return n_hoisted


def build_program():
    p = Program()
    nc = p.build()
    return nc, p


_cache = {}


def _get_nc():
    if "nc" not in _cache:
        _cache["nc"] = build_program()[0]
    return _cache["nc"]


def kernel(message, _trace=False):
    """Full (2000000, 16) f32 in -> (2000000, 8) f32 out, 8-core data parallel."""
    from concourse.bass_utils import run_bass_kernel_spmd
    msg = np.asarray(message, dtype=np.float32)
    nc = _get_nc()
    pad = PAD_ROWS - msg.shape[0]
    msgp = np.concatenate([msg, np.zeros((pad, 16), np.float32)]) if pad > 0 else msg
    # per-core column-major fp16: [core, 16, CORE_ROWS]
    perm = [0, 2, 4, 6, 8, 10, 12, 14, 1, 3, 5, 7, 9, 11, 13, 15]
    shards = np.ascontiguousarray(
        msgp.reshape(N_CORES, CORE_ROWS, 16).transpose(0, 2, 1)[:, perm, :]).astype(NPDT)
    in_maps = [{"msgT": shards[i]} for i in range(N_CORES)]
    kw = dict(trace=True) if _trace else {}
    res = run_bass_kernel_spmd(nc, in_maps, core_ids=list(range(N_CORES)), **kw)
    outT = np.stack([res.results[i]["outT"] for i in range(N_CORES)])  # [8, 8, CORE_ROWS]
    out = outT.transpose(0, 2, 1).reshape(PAD_ROWS, 8).astype(np.float32)
    if _trace:
        _cache["last_result"] = res
    return np.ascontiguousarray(out[: msg.shape[0]])
